# revision 1
# baseline (speedup 1.0000x reference)
"""Trainium2 Bass kernel for nn_AutoregressiveDecoder (LSTM decoder w/ greedy sampling).

Strategy (8 NeuronCores, SPMD):
  - Vocab-shard the fc projection: core j holds Wfc rows [4000j, 4000(j+1)) padded to
    4096 (pad bias = -1e30), resident in SBUF in fp32.
  - LSTM weights replicated per core, SBUF-resident, gate columns permuted so that
    PE column-group q computes [i|f|g|o] for hidden quarter q -> full-partition
    elementwise state updates.
  - All matmuls fp32 with 4-way PE column tiling (tile_position=(0,32g)).
  - Greedy token each step: per-partition max8/max_index, PE-transpose fold over the
    4 partition groups, 256B AllGather of (val, idx) across cores, arithmetic
    first-occurrence argmax fold, then indirect-DMA gather of the embedding row.
  - Logits stream to DRAM as [T, 128, 1024] per core; host reassembles [B, T, V].
"""
import sys

sys.path.insert(0, "/opt/trn_rl_repo")

import numpy as np

import concourse.bass as bass
import concourse.bacc as bacc
import concourse.tile as tile
import concourse.mybir as mybir
from concourse.bass_utils import run_bass_kernel_spmd

FP32 = mybir.dt.float32
BF16 = mybir.dt.bfloat16
I32 = mybir.dt.int32
U32 = mybir.dt.uint32

N_CORES = 8
B, L, H, E, V, T = 32, 256, 512, 512, 32000, 64
VS = V // N_CORES          # 4000 true shard
VSP = 4096                 # padded shard
BIG = 65536.0

Sigmoid = mybir.ActivationFunctionType.Sigmoid
Tanh = mybir.ActivationFunctionType.Tanh
Alu = mybir.AluOpType


def build_decoder(nc, io, n_steps):
    """Emit the full unrolled decoder. io: dict name -> DRAM AP."""
    with tile.TileContext(nc) as tc:
        sb = tc.alloc_tile_pool(name="sb", bufs=1)
        sb2 = tc.alloc_tile_pool(name="sb2", bufs=3)
        ps_g = tc.alloc_tile_pool(name="ps_g", bufs=2, space="PSUM")
        ps_v = tc.alloc_tile_pool(name="ps_v", bufs=2, space="PSUM")
        ps_s = tc.alloc_tile_pool(name="ps_s", bufs=2, space="PSUM")
        dr = tc.alloc_tile_pool(name="dr", bufs=4, space="DRAM")
        pools = [sb, sb2, ps_g, ps_v, ps_s, dr]

        # ---- persistent SBUF state & weights ----
        # big matmuls run as bf16x3 (hi*hi + hi*lo + lo*hi, fp32 psum accumulate):
        # ~fp32 precision at 1 cycle/row instead of fp32's 4 cycles/row.
        wfh = [sb.tile([128, VSP], BF16, name=f"wfh{q}") for q in range(4)]
        wfl = [sb.tile([128, VSP], BF16, name=f"wfl{q}") for q in range(4)]
        wgh = [sb.tile([128, 2048], BF16, name=f"wgh{r}") for r in range(8)]
        wgl = [sb.tile([128, 2048], BF16, name=f"wgl{r}") for r in range(8)]
        bias_g = sb.tile([1, 2048], FP32, name="bias_g")
        bias_fcv = sb.tile([128, 1024], FP32, name="bias_fcv")
        goffs = sb.tile([128, 1], FP32, name="goffs")
        ident = sb.tile([128, 128], FP32, name="ident")
        ones = sb.tile([1, 32], FP32, name="ones")
        xT = sb.tile([128, 128], FP32, name="xT")
        hT = sb.tile([128, 128], FP32, name="hT")
        xTh = sb.tile([128, 128], BF16, name="xTh")
        xTl = sb.tile([128, 128], BF16, name="xTl")
        hTh = sb.tile([128, 128], BF16, name="hTh")
        hTl = sb.tile([128, 128], BF16, name="hTl")
        c_t = sb.tile([128, 128], FP32, name="c_t")

        for q in range(4):
            nc.sync.dma_start(wfh[q][:], io["wfc_hi"][128 * q:128 * (q + 1), :])
            nc.sync.dma_start(wfl[q][:], io["wfc_lo"][128 * q:128 * (q + 1), :])
        for r in range(8):
            nc.sync.dma_start(wgh[r][:], io["wgates_hi"][128 * r:128 * (r + 1), :])
            nc.sync.dma_start(wgl[r][:], io["wgates_lo"][128 * r:128 * (r + 1), :])
        nc.sync.dma_start(bias_g[:], io["bias_g"][:])
        nc.sync.dma_start(bias_fcv[:], io["bias_fcv"][:])
        nc.sync.dma_start(goffs[:], io["goffs"][:])
        nc.sync.dma_start(ident[:], io["ident"][:])
        nc.sync.dma_start(ones[:], io["ones"][:])
        nc.sync.dma_start(xT[:], io["h0t"][:])
        nc.sync.dma_start(hT[:], io["h0t"][:])
        nc.sync.dma_start(c_t[:], io["c0"][:])
        # initial hi/lo splits of the (identical) x0 = h0 state
        nc.vector.tensor_copy(hTh[:], hT[:])
        nc.vector.tensor_tensor(hTl[:], hT[:], hTh[:], op=Alu.subtract)
        nc.vector.tensor_copy(xTh[:], hTh[:])
        nc.vector.tensor_copy(xTl[:], hTl[:])

        emb = io["emb"]
        out_logits = io["logits"]  # [T, 128, 1024]

        # ---- gates matmul emission helpers (software-pipelined) ----
        # psum layout: partition 32q+b, free = gate*128+hw (cols permuted on host)
        def emit_gates_bias_h(pg):
            for g in range(4):
                nc.tensor.matmul(
                    pg[32 * g:32 * (g + 1), :], lhsT=ones[:, :],
                    rhs=bias_g[:, 512 * g:512 * (g + 1)],
                    start=True, stop=False, tile_position=(0, 32 * g),
                    skip_group_check=True,
                )
            emit_gates_rounds(pg, [4, 5, 6, 7], stop=False)

        def emit_gates_rounds(pg, rounds, stop):
            for r in rounds:
                hi, lo = (xTh, xTl) if r < 4 else (hTh, hTl)
                q = r % 4
                cs = slice(32 * q, 32 * (q + 1))
                for g in range(4):
                    gs = slice(512 * g, 512 * (g + 1))
                    out = pg[32 * g:32 * (g + 1), :]
                    passes = ((hi[:, cs], wgh[r][:, gs]),
                              (lo[:, cs], wgh[r][:, gs]),
                              (hi[:, cs], wgl[r][:, gs]))
                    for pi, (lhsT, rhs) in enumerate(passes):
                        nc.tensor.matmul(
                            out, lhsT=lhsT, rhs=rhs,
                            start=False,
                            stop=(stop and r == rounds[-1] and pi == 2),
                            tile_position=(0, 32 * g),
                            skip_group_check=True,
                        )

        for t in range(n_steps):
            # ================= gates matmul =================
            pg = ps_g.tile([128, 512], FP32, name="pg", tag="pg")
            emit_gates_bias_h(pg)
            emit_gates_rounds(pg, [0, 1, 2, 3], stop=True)

            # ================= activations / state =================
            # gate slots after host permutation: [i | f | o | g(tanh)]
            acts = sb2.tile([128, 512], FP32, name="acts", tag="acts")
            nc.scalar.activation(acts[:, 0:384], pg[:, 0:384], Sigmoid)
            nc.scalar.activation(acts[:, 384:512], pg[:, 384:512], Tanh)
            t1 = sb2.tile([128, 128], FP32, name="t1", tag="t1")
            nc.vector.tensor_tensor(t1[:], acts[:, 0:128], acts[:, 384:512], op=Alu.mult)
            nc.vector.tensor_tensor(c_t[:], acts[:, 128:256], c_t[:], op=Alu.mult)
            nc.vector.tensor_tensor(c_t[:], c_t[:], t1[:], op=Alu.add)
            tanh_c = sb2.tile([128, 128], FP32, name="tanh_c", tag="tanh_c")
            nc.scalar.activation(tanh_c[:], c_t[:], Tanh)
            h_new = sb2.tile([128, 128], FP32, name="h_new", tag="h_new")
            nc.vector.tensor_tensor(h_new[:], acts[:, 256:384], tanh_c[:], op=Alu.mult)

            # hT = transpose(h_new), then hi/lo split for bf16x3
            p_ht = ps_s.tile([128, 128], FP32, name="p_ht", tag="small")
            nc.tensor.transpose(p_ht[:], h_new[:], ident[:])
            nc.scalar.copy(hT[:], p_ht[:])
            nc.vector.tensor_copy(hTh[:], hT[:])
            nc.vector.tensor_tensor(hTl[:], hT[:], hTh[:], op=Alu.subtract)

            # ================= vocab matmul =================
            # psum layout: partition 32g+b (g = vocab quarter of shard), free 1024
            pv = ps_v.tile([128, 1024], FP32, name="pv", tag="pv")
            for nt in range(2):
                for q in range(4):
                    cs = slice(32 * q, 32 * (q + 1))
                    for g in range(4):
                        ws = slice(1024 * g + 512 * nt, 1024 * g + 512 * (nt + 1))
                        out = pv[32 * g:32 * (g + 1), 512 * nt:512 * (nt + 1)]
                        passes = ((hTh[:, cs], wfh[q][:, ws]),
                                  (hTl[:, cs], wfh[q][:, ws]),
                                  (hTh[:, cs], wfl[q][:, ws]))
                        for pi, (lhsT, rhs) in enumerate(passes):
                            nc.tensor.matmul(
                                out, lhsT=lhsT, rhs=rhs,
                                start=(q == 0 and pi == 0),
                                stop=(q == 3 and pi == 2),
                                tile_position=(0, 32 * g),
                                skip_group_check=True,
                            )

            # stage logits to SBUF adding the fc bias (both halves on DVE;
            # logits DMA on the ACT HWDGE ring so the SP ring stays free for
            # the latency-critical exchange DMAs).
            staged = sb2.tile([128, 1024], FP32, name="staged", tag="staged")
            nc.vector.tensor_tensor(staged[:, 0:512], pv[:, 0:512],
                                    bias_fcv[:, 0:512], op=Alu.add)
            nc.vector.tensor_tensor(staged[:, 512:1024], pv[:, 512:1024],
                                    bias_fcv[:, 512:1024], op=Alu.add)
            nc.scalar.dma_start(out_logits[t], staged[:])

            # ================= local argmax =================
            v8 = sb2.tile([128, 8], FP32, name="v8", tag="v8")
            i8 = sb2.tile([128, 8], U32, name="i8", tag="i8")
            nc.vector.max(v8[:], staged[:])
            nc.vector.max_index(i8[:], v8[:], staged[:])
            pay = sb2.tile([128, 2], FP32, name="pay", tag="pay")
            iloc = sb2.tile([128, 1], FP32, name="iloc", tag="iloc")
            nc.vector.tensor_copy(iloc[:], i8[:, 0:1])
            nc.vector.tensor_scalar(pay[:, 1:2], iloc[:], goffs[:, 0:1], None, op0=Alu.add)
            nc.vector.tensor_copy(pay[:, 0:1], v8[:, 0:1])

            # transpose candidates: vboth rows both = vals; payT = (vals, idx)
            p_pa = ps_s.tile([2, 128], FP32, name="p_pa", tag="small")
            nc.tensor.transpose(p_pa[:], pay[:, 0:1].to_broadcast([128, 2]), ident[:])
            vboth = sb2.tile([2, 128], FP32, name="vboth", tag="vboth")
            nc.scalar.copy(vboth[:], p_pa[:])
            p_pb = ps_s.tile([2, 128], FP32, name="p_pb", tag="small")
            nc.tensor.transpose(p_pb[:], pay[:], ident[:])
            payT = sb2.tile([2, 128], FP32, name="payT", tag="payT")
            nc.vector.tensor_copy(payT[:], p_pb[:])  # DVE, parallel to ACT's vboth copy

            # fold over the 4 partition groups (g): first-occurrence argmax
            vb3 = vboth[:].rearrange("p (g b) -> p b g", g=4)
            m4 = sb2.tile([2, 32], FP32, name="m4", tag="m4")
            nc.vector.tensor_reduce(m4[:], vb3, axis=mybir.AxisListType.X, op=Alu.max)
            eq = sb2.tile([2, 128], FP32, name="eq", tag="eq")
            nc.vector.tensor_tensor(
                eq[:].rearrange("p (g b) -> p b g", g=4), vb3,
                m4[:].to_broadcast([2, 32, 4]), op=Alu.is_equal)
            lmi = sb2.tile([2, 128], FP32, name="lmi", tag="lmi")
            nc.vector.tensor_scalar(lmi[:], payT[:], -1.0, BIG, op0=Alu.mult, op1=Alu.add)
            msel = sb2.tile([2, 128], FP32, name="msel", tag="msel")
            nc.vector.tensor_tensor(msel[:], eq[:], lmi[:], op=Alu.mult)
            res = sb2.tile([2, 32], FP32, name="res", tag="res")
            nc.vector.tensor_reduce(
                res[:], msel[:].rearrange("p (g b) -> p b g", g=4),
                axis=mybir.AxisListType.X, op=Alu.max)
            pay2 = sb2.tile([2, 32], FP32, name="pay2", tag="pay2")
            # row 1 (idx): BIG - res, exact for integer idx. row 0 (val): take the
            # exact max from m4 — round-tripping val through BIG-val would quantize
            # it to ulp(BIG)=2^-7 and mis-break near-ties across cores.
            nc.vector.tensor_scalar(pay2[:], res[:], -1.0, BIG,
                                    op0=Alu.mult, op1=Alu.add)
            nc.vector.tensor_copy(pay2[0:1, :], m4[0:1, :])

            # ================= AllGather exchange =================
            cc_in = dr.tile([2, 32], FP32, name="cc_in", tag="cc_in")
            cc_out = dr.tile([16, 32], FP32, name="cc_out", tag="cc_out",
                             addr_space="Shared")
            nc.sync.dma_start(cc_in[:], pay2[:])
            nc.gpsimd.collective_compute(
                "AllGather", Alu.bypass,
                replica_groups=[list(range(N_CORES))],
                ins=[cc_in[:]], outs=[cc_out[:]],
            )
            agb = sb2.tile([16, 32], FP32, name="agb", tag="agb")
            nc.sync.dma_start(agb[:], cc_out[:])

            # transpose [16,32] -> [32,16]: partition = batch
            p_t32 = ps_s.tile([32, 16], FP32, name="p_t32", tag="small")
            nc.tensor.transpose(p_t32[:], agb[:], ident[0:16, 0:16])
            t32 = sb2.tile([32, 16], FP32, name="t32", tag="t32")
            nc.scalar.copy(t32[:], p_t32[:])

            # fold over 8 ranks, first-occurrence (lowest global idx on ties)
            vals = t32[:, 0:16:2]
            idxs = t32[:, 1:16:2]
            gv32 = sb2.tile([32, 1], FP32, name="gv32", tag="gv32")
            nc.vector.tensor_reduce(gv32[:], vals, axis=mybir.AxisListType.X, op=Alu.max)
            eqr = sb2.tile([32, 8], FP32, name="eqr", tag="eqr")
            nc.vector.tensor_scalar(eqr[:], vals, gv32[:, 0:1], None, op0=Alu.is_equal)
            lmir = sb2.tile([32, 8], FP32, name="lmir", tag="lmir")
            nc.vector.tensor_scalar(lmir[:], idxs, -1.0, BIG, op0=Alu.mult, op1=Alu.add)
            mselr = sb2.tile([32, 8], FP32, name="mselr", tag="mselr")
            nc.vector.tensor_tensor(mselr[:], eqr[:], lmir[:], op=Alu.mult)
            m2r = sb2.tile([32, 1], FP32, name="m2r", tag="m2r")
            nc.vector.tensor_reduce(m2r[:], mselr[:], axis=mybir.AxisListType.X, op=Alu.max)
            idxf = sb2.tile([32, 1], FP32, name="idxf", tag="idxf")
            nc.vector.tensor_scalar(idxf[:], m2r[:], -1.0, BIG, op0=Alu.mult, op1=Alu.add)
            idx32 = sb2.tile([32, 1], I32, name="idx32", tag="idx32")
            nc.vector.tensor_copy(idx32[:], idxf[:])

            # ================= embedding gather + transpose =================
            if t < n_steps - 1:
                x_rows = sb2.tile([32, 512], FP32, name="x_rows", tag="x_rows")
                nc.gpsimd.indirect_dma_start(
                    out=x_rows[:], out_offset=None, in_=emb[:],
                    in_offset=bass.IndirectOffsetOnAxis(ap=idx32[:, 0:1], axis=0),
                )
                p_x = ps_s.tile([128, 128], FP32, name="p_x", tag="small")
                for q in range(4):
                    nc.tensor.transpose(
                        p_x[:, 32 * q:32 * (q + 1)],
                        x_rows[:, 128 * q:128 * (q + 1)], ident[0:32, 0:32])
                nc.scalar.copy(xT[:], p_x[:])
                nc.vector.tensor_copy(xTh[:], xT[:])
                nc.vector.tensor_tensor(xTl[:], xT[:], xTh[:], op=Alu.subtract)

        for p in reversed(pools):
            p.release()


def host_prep(inputs):
    """Build per-core in_maps from the full problem inputs."""
    z = np.asarray(inputs["z"], np.float32)
    embedding = np.ascontiguousarray(np.asarray(inputs["embedding"], np.float32))
    Wh = np.asarray(inputs["Wh"], np.float32)
    bh = np.asarray(inputs["bh"], np.float32)
    Wc = np.asarray(inputs["Wc"], np.float32)
    bc = np.asarray(inputs["bc"], np.float32)
    Wih = np.asarray(inputs["Wih"], np.float32)
    Whh = np.asarray(inputs["Whh"], np.float32)
    bih = np.asarray(inputs["bih"], np.float32)
    bhh = np.asarray(inputs["bhh"], np.float32)
    Wfc = np.asarray(inputs["Wfc"], np.float32)
    bfc = np.asarray(inputs["bfc"], np.float32)

    h0 = (z @ Wh.T + bh).astype(np.float32)   # [B, H]
    c0 = (z @ Wc.T + bc).astype(np.float32)
    b_gates = (bih + bhh).astype(np.float32)  # [4H]

    # gate column permutation: c' = q*512 + slot*128 + hw with slot order
    # [i, f, o, g] so the sigmoid gates are one contiguous 384-wide range.
    cp = np.arange(2048)
    qq, rem = cp // 512, cp % 512
    slot, hw = rem // 128, rem % 128
    gate = np.array([0, 1, 3, 2])[slot]        # slot -> original gate (i,f,o,g)
    perm = gate * 512 + qq * 128 + hw          # original col index for permuted col c'
    Wall = np.concatenate([Wih, Whh], axis=1)  # [2048, 1024] (k = [x | h])
    Wperm = Wall[perm]                         # [2048, 1024]
    wgates = np.ascontiguousarray(Wperm.T)     # [1024, 2048]
    bias_g = np.ascontiguousarray(b_gates[perm])[None, :]  # [1, 2048]

    def split_bf16(w):
        import ml_dtypes
        hi = w.astype(ml_dtypes.bfloat16)
        lo = (w - hi.astype(np.float32)).astype(ml_dtypes.bfloat16)
        return np.ascontiguousarray(hi), np.ascontiguousarray(lo)

    wgates_hi, wgates_lo = split_bf16(wgates)

    # state layout tiles
    h0t = np.zeros((128, 128), np.float32)     # h0t[p, q*32+b] = h0[b, 128q+p]
    c0t = np.zeros((128, 128), np.float32)     # c0t[32q+b, hw] = c0[b, 128q+hw]
    for q in range(4):
        h0t[:, 32 * q:32 * (q + 1)] = h0[:, 128 * q:128 * (q + 1)].T
        c0t[32 * q:32 * (q + 1), :] = c0[:, 128 * q:128 * (q + 1)]

    ident = np.eye(128, dtype=np.float32)
    ones = np.ones((1, 32), np.float32)

    in_maps = []
    for j in range(N_CORES):
        shard = Wfc[VS * j:VS * (j + 1)]                    # [4000, 512]
        shard_p = np.zeros((VSP, H), np.float32)
        shard_p[:VS] = shard
        wfc_in = np.ascontiguousarray(shard_p.T)            # [512, 4096]
        wfc_hi, wfc_lo = split_bf16(wfc_in)
        bfc_p = np.full(VSP, -1e30, np.float32)
        bfc_p[:VS] = bfc[VS * j:VS * (j + 1)]
        # staged-layout bias: bias_fcv[32g+b, v] = bfc_p[1024g + v]
        bias_fcv = np.repeat(bfc_p.reshape(4, 1, 1024), 32, axis=1).reshape(128, 1024)
        goffs = (VS * j + (np.arange(128) // 32) * 1024).astype(np.float32)[:, None]
        in_maps.append({
            "wfc_hi": wfc_hi,
            "wfc_lo": wfc_lo,
            "wgates_hi": wgates_hi,
            "wgates_lo": wgates_lo,
            "bias_g": bias_g,
            "bias_fcv": np.ascontiguousarray(bias_fcv),
            "goffs": goffs,
            "ident": ident,
            "ones": ones,
            "h0t": h0t,
            "c0": c0t,
            "emb": embedding,
        })
    return in_maps


def declare_io(nc, n_steps):
    io = {}
    io["wfc_hi"] = nc.dram_tensor("wfc_hi", [512, VSP], BF16, kind="ExternalInput").ap()
    io["wfc_lo"] = nc.dram_tensor("wfc_lo", [512, VSP], BF16, kind="ExternalInput").ap()
    io["wgates_hi"] = nc.dram_tensor("wgates_hi", [1024, 2048], BF16, kind="ExternalInput").ap()
    io["wgates_lo"] = nc.dram_tensor("wgates_lo", [1024, 2048], BF16, kind="ExternalInput").ap()
    io["bias_g"] = nc.dram_tensor("bias_g", [1, 2048], FP32, kind="ExternalInput").ap()
    io["bias_fcv"] = nc.dram_tensor("bias_fcv", [128, 1024], FP32, kind="ExternalInput").ap()
    io["goffs"] = nc.dram_tensor("goffs", [128, 1], FP32, kind="ExternalInput").ap()
    io["ident"] = nc.dram_tensor("ident", [128, 128], FP32, kind="ExternalInput").ap()
    io["ones"] = nc.dram_tensor("ones", [1, 32], FP32, kind="ExternalInput").ap()
    io["h0t"] = nc.dram_tensor("h0t", [128, 128], FP32, kind="ExternalInput").ap()
    io["c0"] = nc.dram_tensor("c0", [128, 128], FP32, kind="ExternalInput").ap()
    io["emb"] = nc.dram_tensor("emb", [V, E], FP32, kind="ExternalInput").ap()
    io["logits"] = nc.dram_tensor("logits", [n_steps, 128, 1024], FP32,
                                  kind="ExternalOutput").ap()
    return io


_BUILT = {}


def build(n_steps=T):
    if n_steps in _BUILT:
        return _BUILT[n_steps]
    nc = bacc.Bacc("TRN2", target_bir_lowering=False, debug=False,
                   num_devices=N_CORES)
    io = declare_io(nc, n_steps)
    build_decoder(nc, io, n_steps)
    nc.compile()
    _BUILT[n_steps] = nc
    return nc


def assemble(results, n_steps=T):
    """results: list of per-core out dicts -> full [B, T, V] fp32."""
    full = np.empty((B, n_steps, V), np.float32)
    for j in range(N_CORES):
        arr = results[j]["logits"].reshape(n_steps, 4, 32, 1024)
        arr = arr.transpose(2, 0, 1, 3).reshape(B, n_steps, VSP)[:, :, :VS]
        full[:, :, VS * j:VS * (j + 1)] = arr
    return full


def kernel(**inputs):
    n_steps = int(inputs.get("context_length", T))
    assert n_steps == T, f"kernel hardcodes T={T}, got {n_steps}"
    nc = build(T)
    in_maps = host_prep(inputs)
    res = run_bass_kernel_spmd(nc, in_maps, core_ids=list(range(N_CORES)))
    return assemble(res.results, T)


if __name__ == "__main__":
    import reference
    inputs = reference.setup_inputs()
    out = kernel(**{k: np.asarray(v) if hasattr(v, "shape") else v
                    for k, v in inputs.items()})
    print("output shape:", out.shape)

